# revision 15
# baseline (speedup 1.0000x reference)
"""BERT self-attention Bass/Tile kernel for 8 Trainium2 NeuronCores.

Problem: hidden [2, 2048, 768], 12 heads x 64 dim, additive mask [2,1,1,2048].
Sharding: batch x head-group. Core c handles batch b = c // 4 and global heads
3*(c%4) .. 3*(c%4)+2 (columns 192*(c%4) .. +192 of Wq/Wk/Wv).  Each core
computes its 3 heads' full attention locally; outputs are concatenated on the
host (no cross-device communication).

Engine model (measured on HW): the PE streams matmul output columns at
1 col/cycle (2.4 GHz warm; the clock gate holds 1.2 GHz for the first ~28us
of every launch regardless of activity, so warmup spinning is useless).  The
PE is the binding engine; the big levers used here:

  - X loads are gpsimd SWDGE casting DMAs (DRAM f32 -> SBUF fp16 directly);
    W loads ride the sync HWDGE queue in parallel with a DVE cast.
  - Score matmuls contract over K=64 (head dim) = half the PE array.  Each
    score tile [128k, 1024] holds TWO 512-wide halves computed by two matmuls
    on DISJOINT row groups of the array (operands at partitions 0:64 vs
    64:128, row placement auto-derived from base partition), which the PE
    runs CONCURRENTLY: ~2x on the dominant score cost.  h0/h1 pair naturally;
    for h2 the q2/k2 rows are duplicated into partitions 64:128 by a small
    SBUF->SBUF DMA shift.  Sharing ONE PSUM tile per pair stops the list
    scheduler from splitting the pair.

Per-core pipeline (one TileContext):
  X [2048,768] --gpsimd cast-DMA--> x16 fp16 --PE transpose--> X_T [768,2048]
  Q_T/K_T pairs = W.T @ X_T  (heads 0,1 packed M=128)             (24.6k cols)
  [q_h2|k_h2]  = one M=128 matmul vs combined weight tile         (12.3k cols)
  V directly in [k, d] layout: X_T chunk stationary, Wv moving,
     bias via a K=1 ones-row matmul; no V transpose               (21.5k cols)
  score tile (J, kc | J, t): 2 concurrent N=512 matmuls K=64      (98.3k cols,
     ~2x row-tile concurrency)
  probs = exp(scores/8): ONE activation per tile (ScalarE Exp or DVE
     Schraudolph fast-exp), fp16
  ctx chains (h, J, s): 16 x N=65 matmuls accumulate probs.T @ V_aug;
     col 64 = softmax denominator (e-column of V_aug)             (49.9k cols)
  out[q, d] = ctx[:, :64] * (1 / ctx[:, 64])   -> DMA to DRAM

The additive mask folds into V: exp(s + m_k) = exp(s) * exp(m_k); both the
numerator and denominator columns of V_aug are pre-scaled by exp(m_k) (a
per-partition scalar in the [k, d] layout).  All-zero mask skips the scale
and memsets the denominator column to 1.
"""

import numpy as np

import concourse.bass as bass
import concourse.tile as tile
from concourse import bacc, mybir
from concourse.bass_utils import run_bass_kernel_spmd
from concourse.masks import make_identity

F32 = mybir.dt.float32
F16 = mybir.dt.float16
EXP = mybir.ActivationFunctionType.Exp
COPY = mybir.ActivationFunctionType.Copy

S = 2048           # sequence length
DM = 768           # model dim
DH = 64            # head dim
NHL = 3            # local heads per core
FC = DM // 128     # 6 f-chunks (contraction for projections)
KC = S // 128      # 16 k-chunks
QB = 512           # q block width (J indexes blocks of 512)
NQB = S // QB      # 4 q blocks


def _build_kernel(zero_mask: bool, zero_bias: bool) -> bass.Bass:
    nc = bacc.Bacc()

    x_d = nc.declare_dram_parameter("x", [S, DM], F32, isOutput=False)
    wq_d = nc.declare_dram_parameter("wq", [DM, 192], F32, isOutput=False)
    wk_d = nc.declare_dram_parameter("wk", [DM, 192], F32, isOutput=False)
    wv_d = nc.declare_dram_parameter("wv", [DM, 192], F32, isOutput=False)
    bq_d = nc.declare_dram_parameter("bq", [192], F32, isOutput=False)
    bk_d = nc.declare_dram_parameter("bk", [192], F32, isOutput=False)
    bv_d = nc.declare_dram_parameter("bv", [192], F32, isOutput=False)
    m_d = nc.declare_dram_parameter("mask", [S], F32, isOutput=False)
    out_d = nc.declare_dram_parameter("out", [S, 192], F32, isOutput=True)

    with tile.TileContext(nc) as tc:
        _attention(tc, x_d, (wq_d, wk_d, wv_d), (bq_d, bk_d, bv_d), m_d, out_d,
                   zero_mask, zero_bias)
    nc.compile()
    return nc


# Schraudolph fast-exp on the DVE: fp16 bits of 2^u are ~1024*(u+15)+m with a
# piecewise-linear mantissa, so int16(round(s*A + B)) bit-viewed as fp16
# approximates exp(s/8) to ~1.8% RMS (shift 29 minimizes the multiplicative
# residual; the mean bias cancels in the softmax normalize).  Raw scores are
# N(0,64): t stays in [6.5k, 24k], far from int16/fp16 range edges.  Offloading
# ~1/3 of the exp tiles to the DVE splits the softmax wall across two engines;
# the error hits only those tiles' outputs: ~0.017*sqrt(32/96) ~ 1e-2 global.
FAST_A = float(0.125 * np.log2(np.e) * 1024)   # 184.665
FAST_B = float(15 * 1024 - 29)


def _attention(tc, x_d, w_ds, b_ds, m_d, out_d, zero_mask, zero_bias):
    nc = tc.nc

    const = tc.alloc_tile_pool(name="const", bufs=1)
    xpool = tc.alloc_tile_pool(name="xpool", bufs=5)
    persist = tc.alloc_tile_pool(name="persist", bufs=1)
    probs_pool = tc.alloc_tile_pool(name="probs", bufs=36)
    small = tc.alloc_tile_pool(name="small", bufs=4)
    outp = tc.alloc_tile_pool(name="outp", bufs=1)
    ps = tc.alloc_tile_pool(name="ps", bufs=2, space="PSUM")

    # --- persistent projection outputs --------------------------------------
    # QT2/KT2: [128, 2048] fp16, rows 0:64 = head0, 64:128 = head1
    # QTs/KTs: head2 duplicated at rows 0:64 AND 64:128 so h2 score tiles can
    # use both halves of the PE array (row-tile pairing).
    # XTbig packs the 6 f-chunks of X^T as [128, f, s] so one strided DVE
    # copy per seq-tile scatters a whole per-j transpose group.
    XTbig = persist.tile([128, FC * S], F16, name="XTbig")
    XTv = XTbig[:, :].rearrange("p (f s) -> p f s", s=S)

    def XTf(f, sl):
        return XTv[:, f, sl]
    QT2 = persist.tile([128, S], F16)
    KT2 = persist.tile([128, S], F16)
    QTs = persist.tile([128, S], F16)
    KTs = persist.tile([128, S], F16)
    # V[kc] layout: [V_h0(64) | e | V_h1(64) | e | V_h2(64) | e], e = exp(m_k)
    V = [persist.tile([128, 195], F16, name=f"V_{kc}") for kc in range(KC)]

    out_tiles = [outp.tile([128, 192], F32, name=f"o_{u}") for u in range(16)]
    out_written = [0] * 16

    def load_x_block(m):
        """gpsimd SWDGE casting DMA: DRAM fp32 -> SBUF fp16 directly."""
        x16s = []
        for j in range(4):
            qt = 4 * m + j
            x16 = xpool.tile([128, DM], F16, name=f"x16_{qt}", tag="x16",
                             bufs=5)
            nc.gpsimd.dma_start(out=x16, in_=x_d[128 * qt:128 * (qt + 1), :])
            x16s.append(x16)
        return x16s

    ident16 = const.tile([128, 128], F16)
    make_identity(nc, ident16)
    # Prime the ScalarE activation table (exp set) during the DMA lead-in so
    # the ~2.7us ACT_TABLE_LOAD doesn't sit on the first score tile's path.
    prime = const.tile([128, 16], F32)
    nc.scalar.activation(prime, ident16[:, 0:16], EXP)
    if not zero_bias:
        ones512 = const.tile([1, 512], F16)
        nc.gpsimd.memset(ones512, 1.0)
        ones_row = const.tile([1, 128], F16)
        nc.gpsimd.memset(ones_row, 1.0)

    # x blocks stream on the gpsimd SWDGE queue (it alone can cast); the W
    # fp32 loads ride the sync HWDGE queue in parallel, cast by the DVE which
    # is otherwise idle this early.
    x16s_first = load_x_block(0)

    w16 = []   # q, k, v -> [128, 1152] fp16, f-chunk f at cols 192f..+192
    for t in (1, 0, 2):
        w32 = small.tile([128, FC * 192], F32, name=f"w32_{t}", tag="w32",
                         bufs=3)
        nc.sync.dma_start(
            out=w32[:, :].rearrange("p (f j) -> p f j", j=192),
            in_=w_ds[t][:, :].rearrange("(f p) j -> p f j", p=128))
        wt = const.tile([128, FC * 192], F16, name=f"w16_{t}")
        nc.vector.tensor_copy(out=wt, in_=w32)
        w16.append(wt)
    w16 = [w16[1], w16[0], w16[2]]  # back to q, k, v order

    wsolo = []
    brow16 = []

    def emit_deferred_consts():
        """Emitted after block-0's Q projection: head-2 combined weights,
        bias rows, mask -- none gate the first score tiles."""
        for f in range(FC):
            st = const.tile([128, 128], F16, name=f"wsolo_{f}")
            nc.vector.tensor_copy(out=st[:, 0:64],
                                  in_=w16[0][:, 192 * f + 128:192 * f + 192])
            nc.vector.tensor_copy(out=st[:, 64:128],
                                  in_=w16[1][:, 192 * f + 128:192 * f + 192])
            wsolo.append(st)
        if not zero_bias:
            for t in range(3):
                b16 = const.tile([1, 192], F16, name=f"b16_{t}")
                nc.gpsimd.dma_start(
                    out=b16, in_=b_ds[t][:].rearrange("(o j) -> o j", o=1))
                brow16.append(b16)
            bs = const.tile([1, 128], F16)  # [bq[128:192] | bk[128:192]]
            nc.vector.tensor_copy(out=bs[:, 0:64], in_=brow16[0][:, 128:192])
            nc.vector.tensor_copy(out=bs[:, 64:128], in_=brow16[1][:, 128:192])
            brow16.append(bs)  # bsolo_row at index 3
        nc.gpsimd.dma_start(out=mask_t,
                            in_=m_d[:].rearrange("(i p) -> p i", p=128))
        nc.scalar.activation(expm, mask_t, EXP)  # also primes the act table

    mask_t = const.tile([128, KC], F32)  # mask[128*i + p] at [p, i]
    expm = const.tile([128, KC], F32)    # exp(mask), per k position

    def transpose_block(m, x16s):
        """PE-transpose the 4 fp16 q-tiles into XTbig[:, f, 512m:512m+512].
        Grouped per SEQ-TILE j (not per f) so transposes start as soon as
        each x16 tile's DMA lands; one strided DVE copy scatters all 6
        f-chunks of a j-group."""
        for j in range(4):
            tp = ps.tile([128, FC * 128], F16, name=f"xt_ps_{m}_{j}",
                         tag="mid", bufs=2)
            for f in range(FC):
                nc.tensor.transpose(
                    tp[:, 128 * f:128 * (f + 1)],
                    x16s[j][:, 128 * f:128 * (f + 1)],
                    ident16,
                )
            lo = 512 * m + 128 * j
            nc.vector.tensor_copy(
                out=XTv[:, :, lo:lo + 128],
                in_=tp[:, :].rearrange("p (f c) -> p f c", c=128))

    def proj_pair(t, dst_pair, m):
        cols = slice(512 * m, 512 * (m + 1))
        pp = ps.tile([128, 512], F32, name=f"proj_{t}_{m}", tag="mid", bufs=2)
        if not zero_bias:
            nc.tensor.matmul(pp, brow16[t][:, 0:128], ones512,
                             start=True, stop=False)
        for f in range(FC):
            nc.tensor.matmul(pp, w16[t][:, 192 * f:192 * f + 128],
                             XTf(f, cols),
                             start=(zero_bias and f == 0), stop=(f == FC - 1))
        nc.vector.tensor_copy(out=dst_pair[:, cols], in_=pp)

    def proj_solo(m):
        """q_h2 (psum rows 0:64) and k_h2 (rows 64:128) in one M=128 chain.
        Both are then duplicated to the other partition half by SBUF->SBUF
        DMA shifts (gpsimd) so h2 score tiles can use either row half."""
        cols = slice(512 * m, 512 * (m + 1))
        sp = ps.tile([128, 512], F32, name=f"proj_s_{m}", tag="mid", bufs=2)
        if not zero_bias:
            nc.tensor.matmul(sp, brow16[3], ones512, start=True, stop=False)
        for f in range(FC):
            nc.tensor.matmul(sp, wsolo[f], XTf(f, cols),
                             start=(zero_bias and f == 0), stop=(f == FC - 1))
        nc.vector.tensor_copy(out=QTs[0:64, cols], in_=sp[0:64])
        nc.vector.tensor_copy(out=KTs[64:128, cols], in_=sp[64:128])
        nc.gpsimd.dma_start(out=KTs[0:64, cols], in_=KTs[64:128, cols])
        nc.gpsimd.dma_start(out=QTs[64:128, cols], in_=QTs[0:64, cols])

    def build_v(kc):
        """V[kc] [k, d] directly: X_T chunks stationary, Wv moving; bias via
        the ones-row K=1 matmul; then fp16 copy (+ mask scale) into V_aug."""
        u = kc  # seq-tile index
        vp = ps.tile([128, 192], F32, name=f"v_ps_{kc}", tag="mid", bufs=2)
        if not zero_bias:
            nc.tensor.matmul(vp, ones_row, brow16[2], start=True, stop=False)
        for f in range(FC):
            nc.tensor.matmul(vp, XTf(f, slice(128 * u, 128 * (u + 1))),
                             w16[2][:, 192 * f:192 * f + 192],
                             start=(zero_bias and f == 0), stop=(f == FC - 1))
        ecol = bass.AP(tensor=V[kc].tensor, offset=V[kc].offset + 64,
                       ap=[V[kc].ap[0], [65, 3]])
        vdst = bass.AP(tensor=V[kc].tensor, offset=V[kc].offset,
                       ap=[V[kc].ap[0], [65, 3], [1, 64]])
        vsrc = vp[:, 0:192].rearrange("p (h d) -> p h d", d=64)
        if zero_mask:
            nc.vector.tensor_copy(out=vdst, in_=vsrc)
            nc.gpsimd.memset(ecol, 1.0)
        else:
            nc.vector.tensor_scalar_mul(out=vdst, in0=vsrc,
                                        scalar1=expm[:, kc:kc + 1])
            esrc = bass.AP(tensor=expm.tensor, offset=expm.offset + kc,
                           ap=[expm.ap[0], [0, 3]])
            nc.vector.tensor_copy(out=ecol, in_=esrc)

    # --- score tiles ---------------------------------------------------------
    # Tile ('h01', J, kc): cols 0:512 = h0 scores [k(kc), q(J)], 512:1024 = h1.
    # Tile ('h2', J, t):   cols 0:512 = h2 kc=2t, 512:1024 = h2 kc=2t+1.
    # Each is TWO concurrent K=64 matmuls on disjoint PE row halves sharing
    # one PSUM tile, then ONE exp.
    def emit_score_tile(kind, J, i, use_dve):
        sc = ps.tile([128, 1024], F32, name=f"sc_{kind}_{J}_{i}", tag="sc",
                     bufs=3)
        if kind == "h01":
            kc = i
            nc.tensor.matmul(sc[:, 0:512],
                             KT2[0:64, 128 * kc:128 * (kc + 1)],
                             QT2[0:64, 512 * J:512 * (J + 1)],
                             start=True, stop=True)
            nc.tensor.matmul(sc[:, 512:1024],
                             KT2[64:128, 128 * kc:128 * (kc + 1)],
                             QT2[64:128, 512 * J:512 * (J + 1)],
                             start=True, stop=True)
        else:
            kca, kcb = 2 * i, 2 * i + 1
            nc.tensor.matmul(sc[:, 0:512],
                             KTs[0:64, 128 * kca:128 * (kca + 1)],
                             QTs[0:64, 512 * J:512 * (J + 1)],
                             start=True, stop=True)
            nc.tensor.matmul(sc[:, 512:1024],
                             KTs[64:128, 128 * kcb:128 * (kcb + 1)],
                             QTs[64:128, 512 * J:512 * (J + 1)],
                             start=True, stop=True)
        pt = probs_pool.tile([128, 1024], F16, name=f"pb_{kind}_{J}_{i}",
                             tag="probs")
        if use_dve:
            nc.vector.tensor_scalar(
                out=pt[:, :].bitcast(mybir.dt.int16), in0=sc,
                scalar1=FAST_A, scalar2=FAST_B,
                op0=mybir.AluOpType.mult, op1=mybir.AluOpType.add)
        else:
            nc.scalar.activation(pt, sc, EXP, scale=0.125)
        return pt

    def ctx_chain(h, J, s):
        """One q-sub-tile's ctx accumulation + normalize + out store."""
        cx = ps.tile([128, 512], F32, name=f"cx_{h}_{J}_{s}", tag="mid", bufs=2)
        for kc in range(KC):
            if h < 2:
                pt = tile_probs[("h01", J, kc)]
                off = 512 * h + 128 * s
            else:
                pt = tile_probs[("h2", J, kc // 2)]
                off = 512 * (kc % 2) + 128 * s
            nc.tensor.matmul(
                cx[:, 0:65], pt[:, off:off + 128], V[kc][:, 65 * h:65 * h + 65],
                start=(kc == 0), stop=(kc == KC - 1))
        r = small.tile([128, 1], F32, name=f"r_{h}_{J}_{s}", tag="recip")
        nc.vector.reciprocal_approx_fast(r, cx[:, 64:65])
        u = 4 * J + s
        nc.vector.tensor_scalar_mul(
            out=out_tiles[u][:, 64 * h:64 * (h + 1)],
            in0=cx[:, 0:64], scalar1=r)
        out_written[u] += 1
        if out_written[u] == NHL:
            nc.sync.dma_start(out=out_d[128 * u:128 * (u + 1), :],
                              in_=out_tiles[u])

    # --- emission ------------------------------------------------------------
    # Steps: ('h01', J) completes when its 16 kc tiles are done -> 8 chains
    # (h0/h1 x 4 s).  ('h2', J) completes with its 8 t tiles -> 4 chains.
    tiles = []
    for J in range(NQB):
        for kc in range(KC):
            tiles.append(("h01", J, kc))
        for t in range(KC // 2):
            tiles.append(("h2", J, t))
    STEP_SIZE = {"h01": KC, "h2": KC // 2}
    STEP_CHAINS = {"h01": [(0, 0), (1, 0), (0, 1), (1, 1), (0, 2), (1, 2),
                           (0, 3), (1, 3)],   # (h, s)
                   "h2": [(2, 0), (2, 1), (2, 2), (2, 3)]}
    tile_probs = {}
    emitted = set()
    pending = []          # ctx chains ready to emit
    done_tiles = {(k, J): 0 for k in ("h01", "h2") for J in range(NQB)}
    chains_done = {(k, J): 0 for k in ("h01", "h2") for J in range(NQB)}
    vdone = [False]
    exp_count = [0]
    live_probs = [0]      # probs tiles not yet released by their last chain

    def queue_step_chains(kind, J):
        for (h, s) in STEP_CHAINS[kind]:
            pending.append((h, J, s))

    def emit_tile(key):
        kind, J, i = key
        c = exp_count[0]
        exp_count[0] += 1
        tile_probs[key] = emit_score_tile(kind, J, i,
                                          use_dve=(c % 7 in (1, 4)))
        emitted.add(key)
        live_probs[0] += 1
        done_tiles[(kind, J)] += 1
        if done_tiles[(kind, J)] == STEP_SIZE[kind] and vdone[0]:
            queue_step_chains(kind, J)

    def emit_chain(h, J, s):
        ctx_chain(h, J, s)
        kind = "h01" if h < 2 else "h2"
        chains_done[(kind, J)] += 1
        if chains_done[(kind, J)] == len(STEP_CHAINS[kind]):
            live_probs[0] -= STEP_SIZE[kind]

    def tile_ready(key, q_m, k_m, solo_m):
        kind, J, i = key
        if kind == "h01":
            return J <= q_m and i <= 4 * k_m + 3
        return J <= solo_m and 2 * i + 1 <= 4 * solo_m + 3

    def try_emit(q_m, k_m, solo_m, budget):
        # cap pre-vdone emissions below the probs pool depth: a tile whose
        # probs slot needs a ctx chain to free it deadlocks the PE before V
        # is built (chains are gated on vdone).
        for key in tiles:
            if budget <= 0 or live_probs[0] >= 32:
                return
            if key in emitted:
                continue
            if tile_ready(key, q_m, k_m, solo_m):
                emit_tile(key)
                budget -= 1

    for m in range(4):
        x16s = x16s_first if m == 0 else load_x_block(m)
        transpose_block(m, x16s)
        proj_pair(1, KT2, m)   # K first: tile ('h01', J, kc) needs kc block
        try_emit(m - 1, m, m - 1, 4)
        proj_pair(0, QT2, m)
        try_emit(m, m, m - 1, 4)
        if m == 0:
            emit_deferred_consts()
        proj_solo(m)
        try_emit(m, m, m, 4)
        for kc in range(4 * m, 4 * m + 4):
            build_v(kc)
        try_emit(m, m, m, 4)
    vdone[0] = True
    for kind in ("h01", "h2"):
        for J in range(NQB):
            if done_tiles[(kind, J)] == STEP_SIZE[kind]:
                queue_step_chains(kind, J)

    # Tail: remaining tiles interleaved with ctx chains of completed steps.
    # A tile needs a free probs slot (pool holds 36): drain chains until the
    # live count leaves headroom, plus extra pops when the backlog piles up.
    for key in tiles:
        if key in emitted:
            continue
        if pending:
            emit_chain(*pending.pop(0))
        if len(pending) > 4:
            emit_chain(*pending.pop(0))
        while live_probs[0] >= 34 and pending:
            emit_chain(*pending.pop(0))
        emit_tile(key)
    while pending:
        emit_chain(*pending.pop(0))

    for p in (ps, outp, small, probs_pool, persist, xpool, const):
        p.release()


_NC_CACHE = {}


def _get_nc(zero_mask: bool, zero_bias: bool):
    key = (zero_mask, zero_bias)
    if key not in _NC_CACHE:
        _NC_CACHE[key] = _build_kernel(zero_mask, zero_bias)
    return _NC_CACHE[key]


def kernel(hidden_states, attention_mask, Wq, bq, Wk, bk, Wv, bv, **run_kw):
    hidden_states = np.asarray(hidden_states, dtype=np.float32)
    attention_mask = np.asarray(attention_mask, dtype=np.float32)
    Wq, Wk, Wv = (np.asarray(a, dtype=np.float32) for a in (Wq, Wk, Wv))
    bq, bk, bv = (np.asarray(a, dtype=np.float32) for a in (bq, bk, bv))

    zero_mask = bool(np.all(attention_mask == 0.0))
    zero_bias = bool(np.all(bq == 0.0) and np.all(bk == 0.0)
                     and np.all(bv == 0.0))
    nc = _get_nc(zero_mask, zero_bias)
    in_maps = []
    for c in range(8):
        b, g = c // 4, c % 4
        cols = slice(192 * g, 192 * (g + 1))
        in_maps.append({
            "x": np.ascontiguousarray(hidden_states[b]),
            "wq": np.ascontiguousarray(Wq[:, cols]),
            "wk": np.ascontiguousarray(Wk[:, cols]),
            "wv": np.ascontiguousarray(Wv[:, cols]),
            "bq": np.ascontiguousarray(bq[cols]),
            "bk": np.ascontiguousarray(bk[cols]),
            "bv": np.ascontiguousarray(bv[cols]),
            "mask": np.ascontiguousarray(
                np.broadcast_to(attention_mask[b, 0, 0], (S,))),
        })
    res = run_bass_kernel_spmd(nc, in_maps, list(range(8)), **run_kw)
    out = np.empty((2, S, DM), dtype=np.float32)
    for c in range(8):
        b, g = c // 4, c % 4
        out[b, :, 192 * g:192 * (g + 1)] = res.results[c]["out"]
    if run_kw:
        return out, res
    return out


# revision 16
# speedup vs baseline: 1.1404x; 1.1404x over previous
"""BERT self-attention Bass/Tile kernel for 8 Trainium2 NeuronCores.

Problem: hidden [2, 2048, 768], 12 heads x 64 dim, additive mask [2,1,1,2048].
Sharding: batch x head-group. Core c handles batch b = c // 4 and global heads
3*(c%4) .. 3*(c%4)+2 (columns 192*(c%4) .. +192 of Wq/Wk/Wv).  Each core
computes its 3 heads' full attention locally; outputs are concatenated on the
host (no cross-device communication).

Engine model (measured on HW): the PE streams matmul output columns at
1 col/cycle (2.4 GHz warm; the clock gate holds 1.2 GHz for the first ~28us
of every launch regardless of activity, so warmup spinning is useless).  The
PE is the binding engine; the big levers used here:

  - X loads are gpsimd SWDGE casting DMAs (DRAM f32 -> SBUF fp16 directly);
    W loads ride the sync HWDGE queue in parallel with a DVE cast.
  - Score matmuls contract over K=64 (head dim) = half the PE array.  Each
    score tile [128k, 1024] holds TWO 512-wide halves computed by two matmuls
    on DISJOINT row groups of the array (operands at partitions 0:64 vs
    64:128, row placement auto-derived from base partition), which the PE
    runs CONCURRENTLY: ~2x on the dominant score cost.  h0/h1 pair naturally;
    for h2 the q2/k2 rows are duplicated into partitions 64:128 by a small
    SBUF->SBUF DMA shift.  Sharing ONE PSUM tile per pair stops the list
    scheduler from splitting the pair.

Per-core pipeline (one TileContext):
  X [2048,768] --gpsimd cast-DMA--> x16 fp16 --PE transpose--> X_T [768,2048]
  Q_T/K_T pairs = W.T @ X_T  (heads 0,1 packed M=128)             (24.6k cols)
  [q_h2|k_h2]  = one M=128 matmul vs combined weight tile         (12.3k cols)
  V directly in [k, d] layout: X_T chunk stationary, Wv moving,
     bias via a K=1 ones-row matmul; no V transpose               (21.5k cols)
  score tile (J, kc | J, t): 2 concurrent N=512 matmuls K=64      (98.3k cols,
     ~2x row-tile concurrency)
  probs = exp(scores/8): ONE activation per tile (ScalarE Exp or DVE
     Schraudolph fast-exp), fp16
  ctx chains (h, J, s): 16 x N=65 matmuls accumulate probs.T @ V_aug;
     col 64 = softmax denominator (e-column of V_aug)             (49.9k cols)
  out[q, d] = ctx[:, :64] * (1 / ctx[:, 64])   -> DMA to DRAM

The additive mask folds into V: exp(s + m_k) = exp(s) * exp(m_k); both the
numerator and denominator columns of V_aug are pre-scaled by exp(m_k) (a
per-partition scalar in the [k, d] layout).  All-zero mask skips the scale
and memsets the denominator column to 1.
"""

import numpy as np

import concourse.bass as bass
import concourse.tile as tile
from concourse import bacc, mybir
from concourse.bass_utils import run_bass_kernel_spmd
from concourse.masks import make_identity

F32 = mybir.dt.float32
F16 = mybir.dt.float16
EXP = mybir.ActivationFunctionType.Exp
COPY = mybir.ActivationFunctionType.Copy

S = 2048           # sequence length
DM = 768           # model dim
DH = 64            # head dim
NHL = 3            # local heads per core
FC = DM // 128     # 6 f-chunks (contraction for projections)
KC = S // 128      # 16 k-chunks
QB = 512           # q block width (J indexes blocks of 512)
NQB = S // QB      # 4 q blocks


def _build_kernel(zero_mask: bool, zero_bias: bool) -> bass.Bass:
    nc = bacc.Bacc()

    x_d = nc.declare_dram_parameter("x", [S, DM], F32, isOutput=False)
    wq_d = nc.declare_dram_parameter("wq", [DM, 192], F32, isOutput=False)
    wk_d = nc.declare_dram_parameter("wk", [DM, 192], F32, isOutput=False)
    wv_d = nc.declare_dram_parameter("wv", [DM, 192], F32, isOutput=False)
    bq_d = nc.declare_dram_parameter("bq", [192], F32, isOutput=False)
    bk_d = nc.declare_dram_parameter("bk", [192], F32, isOutput=False)
    bv_d = nc.declare_dram_parameter("bv", [192], F32, isOutput=False)
    m_d = nc.declare_dram_parameter("mask", [S], F32, isOutput=False)
    out_d = nc.declare_dram_parameter("out", [S, 192], F32, isOutput=True)

    with tile.TileContext(nc) as tc:
        _attention(tc, x_d, (wq_d, wk_d, wv_d), (bq_d, bk_d, bv_d), m_d, out_d,
                   zero_mask, zero_bias)
    nc.compile()
    return nc


# Schraudolph fast-exp on the DVE: fp16 bits of 2^u are ~1024*(u+15)+m with a
# piecewise-linear mantissa, so int16(round(s*A + B)) bit-viewed as fp16
# approximates exp(s/8) to ~1.8% RMS (shift 29 minimizes the multiplicative
# residual; the mean bias cancels in the softmax normalize).  Raw scores are
# N(0,64): t stays in [6.5k, 24k], far from int16/fp16 range edges.  Offloading
# ~1/3 of the exp tiles to the DVE splits the softmax wall across two engines;
# the error hits only those tiles' outputs: ~0.017*sqrt(32/96) ~ 1e-2 global.
FAST_A = float(0.125 * np.log2(np.e) * 1024)   # 184.665
FAST_B = float(15 * 1024 - 29)


def _attention(tc, x_d, w_ds, b_ds, m_d, out_d, zero_mask, zero_bias):
    nc = tc.nc

    const = tc.alloc_tile_pool(name="const", bufs=1)
    xpool = tc.alloc_tile_pool(name="xpool", bufs=5)
    persist = tc.alloc_tile_pool(name="persist", bufs=1)
    probs_pool = tc.alloc_tile_pool(name="probs", bufs=36)
    small = tc.alloc_tile_pool(name="small", bufs=4)
    outp = tc.alloc_tile_pool(name="outp", bufs=1)
    ps = tc.alloc_tile_pool(name="ps", bufs=2, space="PSUM")

    # --- persistent projection outputs --------------------------------------
    # QT2/KT2: [128, 2048] fp16, rows 0:64 = head0, 64:128 = head1
    # QTs/KTs: head2 duplicated at rows 0:64 AND 64:128 so h2 score tiles can
    # use both halves of the PE array (row-tile pairing).
    XT = [persist.tile([128, S], F16, name=f"XT_{f}") for f in range(FC)]

    def XTf(f, sl):
        return XT[f][:, sl]
    QT2 = persist.tile([128, S], F16)
    KT2 = persist.tile([128, S], F16)
    QTs = persist.tile([128, S], F16)
    KTs = persist.tile([128, S], F16)
    # V[kc] layout: [V_h0(64) | e | V_h1(64) | e | V_h2(64) | e], e = exp(m_k)
    V = [persist.tile([128, 195], F16, name=f"V_{kc}") for kc in range(KC)]

    out_tiles = [outp.tile([128, 192], F32, name=f"o_{u}") for u in range(16)]
    out_written = [0] * 16

    def load_x_block(m):
        """gpsimd SWDGE casting DMA: DRAM fp32 -> SBUF fp16 directly."""
        x16s = []
        for j in range(4):
            qt = 4 * m + j
            x16 = xpool.tile([128, DM], F16, name=f"x16_{qt}", tag="x16",
                             bufs=5)
            nc.gpsimd.dma_start(out=x16, in_=x_d[128 * qt:128 * (qt + 1), :])
            x16s.append(x16)
        return x16s

    ident16 = const.tile([128, 128], F16)
    make_identity(nc, ident16)
    # Prime the ScalarE activation table (exp set) during the DMA lead-in so
    # the ~2.7us ACT_TABLE_LOAD doesn't sit on the first score tile's path.
    prime = const.tile([128, 16], F32)
    nc.scalar.activation(prime, ident16[:, 0:16], EXP)
    if not zero_bias:
        ones512 = const.tile([1, 512], F16)
        nc.gpsimd.memset(ones512, 1.0)
        ones_row = const.tile([1, 128], F16)
        nc.gpsimd.memset(ones_row, 1.0)

    # x blocks stream on the gpsimd SWDGE queue (it alone can cast); the W
    # fp32 loads ride the sync HWDGE queue in parallel, cast by the DVE which
    # is otherwise idle this early.
    x16s_first = load_x_block(0)

    w16 = []   # q, k, v -> [128, 1152] fp16, f-chunk f at cols 192f..+192
    for t in (1, 0, 2):
        w32 = small.tile([128, FC * 192], F32, name=f"w32_{t}", tag="w32",
                         bufs=3)
        nc.sync.dma_start(
            out=w32[:, :].rearrange("p (f j) -> p f j", j=192),
            in_=w_ds[t][:, :].rearrange("(f p) j -> p f j", p=128))
        wt = const.tile([128, FC * 192], F16, name=f"w16_{t}")
        nc.vector.tensor_copy(out=wt, in_=w32)
        w16.append(wt)
    w16 = [w16[1], w16[0], w16[2]]  # back to q, k, v order

    wsolo = []
    brow16 = []

    def emit_deferred_consts():
        """Emitted after block-0's Q projection: head-2 combined weights,
        bias rows, mask -- none gate the first score tiles."""
        for f in range(FC):
            st = const.tile([128, 128], F16, name=f"wsolo_{f}")
            nc.vector.tensor_copy(out=st[:, 0:64],
                                  in_=w16[0][:, 192 * f + 128:192 * f + 192])
            nc.vector.tensor_copy(out=st[:, 64:128],
                                  in_=w16[1][:, 192 * f + 128:192 * f + 192])
            wsolo.append(st)
        if not zero_bias:
            for t in range(3):
                b16 = const.tile([1, 192], F16, name=f"b16_{t}")
                nc.gpsimd.dma_start(
                    out=b16, in_=b_ds[t][:].rearrange("(o j) -> o j", o=1))
                brow16.append(b16)
            bs = const.tile([1, 128], F16)  # [bq[128:192] | bk[128:192]]
            nc.vector.tensor_copy(out=bs[:, 0:64], in_=brow16[0][:, 128:192])
            nc.vector.tensor_copy(out=bs[:, 64:128], in_=brow16[1][:, 128:192])
            brow16.append(bs)  # bsolo_row at index 3
        nc.gpsimd.dma_start(out=mask_t,
                            in_=m_d[:].rearrange("(i p) -> p i", p=128))
        nc.scalar.activation(expm, mask_t, EXP)  # also primes the act table

    mask_t = const.tile([128, KC], F32)  # mask[128*i + p] at [p, i]
    expm = const.tile([128, KC], F32)    # exp(mask), per k position

    def transpose_block(m, x16s):
        """PE-transpose the 4 fp16 q-tiles into XT[f][:, 512m:512m+512].
        PSUM->SBUF copies alternate DVE / ScalarE to split the load."""
        for f in range(FC):
            tp = ps.tile([128, 512], F16, name=f"xt_ps_{m}_{f}", tag="mid",
                         bufs=2)
            for j in range(4):
                nc.tensor.transpose(
                    tp[:, 128 * j:128 * (j + 1)],
                    x16s[j][:, 128 * f:128 * (f + 1)],
                    ident16,
                )
            dst = XT[f][:, 512 * m:512 * (m + 1)]
            if f % 2 == 0:
                nc.vector.tensor_copy(out=dst, in_=tp)
            else:
                nc.scalar.activation(dst, tp, COPY)

    def proj_pair(t, dst_pair, m):
        cols = slice(512 * m, 512 * (m + 1))
        pp = ps.tile([128, 512], F32, name=f"proj_{t}_{m}", tag="mid", bufs=2)
        if not zero_bias:
            nc.tensor.matmul(pp, brow16[t][:, 0:128], ones512,
                             start=True, stop=False)
        for f in range(FC):
            nc.tensor.matmul(pp, w16[t][:, 192 * f:192 * f + 128],
                             XTf(f, cols),
                             start=(zero_bias and f == 0), stop=(f == FC - 1))
        nc.vector.tensor_copy(out=dst_pair[:, cols], in_=pp)

    def proj_solo(m):
        """q_h2 (psum rows 0:64) and k_h2 (rows 64:128) in one M=128 chain.
        Both are then duplicated to the other partition half by SBUF->SBUF
        DMA shifts (gpsimd) so h2 score tiles can use either row half."""
        cols = slice(512 * m, 512 * (m + 1))
        sp = ps.tile([128, 512], F32, name=f"proj_s_{m}", tag="mid", bufs=2)
        if not zero_bias:
            nc.tensor.matmul(sp, brow16[3], ones512, start=True, stop=False)
        for f in range(FC):
            nc.tensor.matmul(sp, wsolo[f], XTf(f, cols),
                             start=(zero_bias and f == 0), stop=(f == FC - 1))
        nc.vector.tensor_copy(out=QTs[0:64, cols], in_=sp[0:64])
        nc.vector.tensor_copy(out=KTs[64:128, cols], in_=sp[64:128])
        nc.gpsimd.dma_start(out=KTs[0:64, cols], in_=KTs[64:128, cols])
        nc.gpsimd.dma_start(out=QTs[64:128, cols], in_=QTs[0:64, cols])

    def build_v(kc):
        """V[kc] [k, d] directly: X_T chunks stationary, Wv moving; bias via
        the ones-row K=1 matmul; then fp16 copy (+ mask scale) into V_aug."""
        u = kc  # seq-tile index
        vp = ps.tile([128, 192], F32, name=f"v_ps_{kc}", tag="mid", bufs=2)
        if not zero_bias:
            nc.tensor.matmul(vp, ones_row, brow16[2], start=True, stop=False)
        for f in range(FC):
            nc.tensor.matmul(vp, XTf(f, slice(128 * u, 128 * (u + 1))),
                             w16[2][:, 192 * f:192 * f + 192],
                             start=(zero_bias and f == 0), stop=(f == FC - 1))
        ecol = bass.AP(tensor=V[kc].tensor, offset=V[kc].offset + 64,
                       ap=[V[kc].ap[0], [65, 3]])
        vdst = bass.AP(tensor=V[kc].tensor, offset=V[kc].offset,
                       ap=[V[kc].ap[0], [65, 3], [1, 64]])
        vsrc = vp[:, 0:192].rearrange("p (h d) -> p h d", d=64)
        if zero_mask:
            nc.vector.tensor_copy(out=vdst, in_=vsrc)
            nc.gpsimd.memset(ecol, 1.0)
        else:
            nc.vector.tensor_scalar_mul(out=vdst, in0=vsrc,
                                        scalar1=expm[:, kc:kc + 1])
            esrc = bass.AP(tensor=expm.tensor, offset=expm.offset + kc,
                           ap=[expm.ap[0], [0, 3]])
            nc.vector.tensor_copy(out=ecol, in_=esrc)

    # --- score tiles ---------------------------------------------------------
    # Tile ('h01', J, kc): cols 0:512 = h0 scores [k(kc), q(J)], 512:1024 = h1.
    # Tile ('h2', J, t):   cols 0:512 = h2 kc=2t, 512:1024 = h2 kc=2t+1.
    # Each is TWO concurrent K=64 matmuls on disjoint PE row halves sharing
    # one PSUM tile, then ONE exp.
    def emit_score_tile(kind, J, i, use_dve):
        sc = ps.tile([128, 1024], F32, name=f"sc_{kind}_{J}_{i}", tag="sc",
                     bufs=3)
        if kind == "h01":
            kc = i
            nc.tensor.matmul(sc[:, 0:512],
                             KT2[0:64, 128 * kc:128 * (kc + 1)],
                             QT2[0:64, 512 * J:512 * (J + 1)],
                             start=True, stop=True)
            nc.tensor.matmul(sc[:, 512:1024],
                             KT2[64:128, 128 * kc:128 * (kc + 1)],
                             QT2[64:128, 512 * J:512 * (J + 1)],
                             start=True, stop=True)
        else:
            kca, kcb = 2 * i, 2 * i + 1
            nc.tensor.matmul(sc[:, 0:512],
                             KTs[0:64, 128 * kca:128 * (kca + 1)],
                             QTs[0:64, 512 * J:512 * (J + 1)],
                             start=True, stop=True)
            nc.tensor.matmul(sc[:, 512:1024],
                             KTs[64:128, 128 * kcb:128 * (kcb + 1)],
                             QTs[64:128, 512 * J:512 * (J + 1)],
                             start=True, stop=True)
        pt = probs_pool.tile([128, 1024], F16, name=f"pb_{kind}_{J}_{i}",
                             tag="probs")
        if use_dve:
            nc.vector.tensor_scalar(
                out=pt[:, :].bitcast(mybir.dt.int16), in0=sc,
                scalar1=FAST_A, scalar2=FAST_B,
                op0=mybir.AluOpType.mult, op1=mybir.AluOpType.add)
        else:
            nc.scalar.activation(pt, sc, EXP, scale=0.125)
        return pt

    def ctx_chain(h, J, s):
        """One q-sub-tile's ctx accumulation + normalize + out store."""
        cx = ps.tile([128, 512], F32, name=f"cx_{h}_{J}_{s}", tag="mid", bufs=2)
        for kc in range(KC):
            if h < 2:
                pt = tile_probs[("h01", J, kc)]
                off = 512 * h + 128 * s
            else:
                pt = tile_probs[("h2", J, kc // 2)]
                off = 512 * (kc % 2) + 128 * s
            nc.tensor.matmul(
                cx[:, 0:65], pt[:, off:off + 128], V[kc][:, 65 * h:65 * h + 65],
                start=(kc == 0), stop=(kc == KC - 1))
        r = small.tile([128, 1], F32, name=f"r_{h}_{J}_{s}", tag="recip")
        nc.vector.reciprocal_approx_fast(r, cx[:, 64:65])
        u = 4 * J + s
        nc.vector.tensor_scalar_mul(
            out=out_tiles[u][:, 64 * h:64 * (h + 1)],
            in0=cx[:, 0:64], scalar1=r)
        out_written[u] += 1
        if out_written[u] == NHL:
            nc.sync.dma_start(out=out_d[128 * u:128 * (u + 1), :],
                              in_=out_tiles[u])

    # --- emission ------------------------------------------------------------
    # Steps: ('h01', J) completes when its 16 kc tiles are done -> 8 chains
    # (h0/h1 x 4 s).  ('h2', J) completes with its 8 t tiles -> 4 chains.
    tiles = []
    for J in range(NQB):
        for kc in range(KC):
            tiles.append(("h01", J, kc))
        for t in range(KC // 2):
            tiles.append(("h2", J, t))
    STEP_SIZE = {"h01": KC, "h2": KC // 2}
    STEP_CHAINS = {"h01": [(0, 0), (1, 0), (0, 1), (1, 1), (0, 2), (1, 2),
                           (0, 3), (1, 3)],   # (h, s)
                   "h2": [(2, 0), (2, 1), (2, 2), (2, 3)]}
    tile_probs = {}
    emitted = set()
    pending = []          # ctx chains ready to emit
    done_tiles = {(k, J): 0 for k in ("h01", "h2") for J in range(NQB)}
    chains_done = {(k, J): 0 for k in ("h01", "h2") for J in range(NQB)}
    vdone = [False]
    exp_count = [0]
    live_probs = [0]      # probs tiles not yet released by their last chain

    def queue_step_chains(kind, J):
        for (h, s) in STEP_CHAINS[kind]:
            pending.append((h, J, s))

    def emit_tile(key):
        kind, J, i = key
        c = exp_count[0]
        exp_count[0] += 1
        tile_probs[key] = emit_score_tile(kind, J, i, use_dve=(c % 3 == 1))
        emitted.add(key)
        live_probs[0] += 1
        done_tiles[(kind, J)] += 1
        if done_tiles[(kind, J)] == STEP_SIZE[kind] and vdone[0]:
            queue_step_chains(kind, J)

    def emit_chain(h, J, s):
        ctx_chain(h, J, s)
        kind = "h01" if h < 2 else "h2"
        chains_done[(kind, J)] += 1
        if chains_done[(kind, J)] == len(STEP_CHAINS[kind]):
            live_probs[0] -= STEP_SIZE[kind]

    def tile_ready(key, q_m, k_m, solo_m):
        kind, J, i = key
        if kind == "h01":
            return J <= q_m and i <= 4 * k_m + 3
        return J <= solo_m and 2 * i + 1 <= 4 * solo_m + 3

    def try_emit(q_m, k_m, solo_m, budget):
        # cap pre-vdone emissions below the probs pool depth: a tile whose
        # probs slot needs a ctx chain to free it deadlocks the PE before V
        # is built (chains are gated on vdone).
        for key in tiles:
            if budget <= 0 or live_probs[0] >= 32:
                return
            if key in emitted:
                continue
            if tile_ready(key, q_m, k_m, solo_m):
                emit_tile(key)
                budget -= 1

    for m in range(4):
        x16s = x16s_first if m == 0 else load_x_block(m)
        transpose_block(m, x16s)
        proj_pair(1, KT2, m)   # K first: tile ('h01', J, kc) needs kc block
        try_emit(m - 1, m, m - 1, 4)
        proj_pair(0, QT2, m)
        try_emit(m, m, m - 1, 4)
        if m == 0:
            emit_deferred_consts()
        proj_solo(m)
        try_emit(m, m, m, 4)
        for kc in range(4 * m, 4 * m + 4):
            build_v(kc)
        try_emit(m, m, m, 4)
    vdone[0] = True
    for kind in ("h01", "h2"):
        for J in range(NQB):
            if done_tiles[(kind, J)] == STEP_SIZE[kind]:
                queue_step_chains(kind, J)

    # Tail: remaining tiles interleaved with ctx chains of completed steps.
    # A tile needs a free probs slot (pool holds 36): drain chains until the
    # live count leaves headroom, plus extra pops when the backlog piles up.
    for key in tiles:
        if key in emitted:
            continue
        if pending:
            emit_chain(*pending.pop(0))
        if len(pending) > 4:
            emit_chain(*pending.pop(0))
        while live_probs[0] >= 34 and pending:
            emit_chain(*pending.pop(0))
        emit_tile(key)
    while pending:
        emit_chain(*pending.pop(0))

    for p in (ps, outp, small, probs_pool, persist, xpool, const):
        p.release()


_NC_CACHE = {}


def _get_nc(zero_mask: bool, zero_bias: bool):
    key = (zero_mask, zero_bias)
    if key not in _NC_CACHE:
        _NC_CACHE[key] = _build_kernel(zero_mask, zero_bias)
    return _NC_CACHE[key]


def kernel(hidden_states, attention_mask, Wq, bq, Wk, bk, Wv, bv, **run_kw):
    hidden_states = np.asarray(hidden_states, dtype=np.float32)
    attention_mask = np.asarray(attention_mask, dtype=np.float32)
    Wq, Wk, Wv = (np.asarray(a, dtype=np.float32) for a in (Wq, Wk, Wv))
    bq, bk, bv = (np.asarray(a, dtype=np.float32) for a in (bq, bk, bv))

    zero_mask = bool(np.all(attention_mask == 0.0))
    zero_bias = bool(np.all(bq == 0.0) and np.all(bk == 0.0)
                     and np.all(bv == 0.0))
    nc = _get_nc(zero_mask, zero_bias)
    in_maps = []
    for c in range(8):
        b, g = c // 4, c % 4
        cols = slice(192 * g, 192 * (g + 1))
        in_maps.append({
            "x": np.ascontiguousarray(hidden_states[b]),
            "wq": np.ascontiguousarray(Wq[:, cols]),
            "wk": np.ascontiguousarray(Wk[:, cols]),
            "wv": np.ascontiguousarray(Wv[:, cols]),
            "bq": np.ascontiguousarray(bq[cols]),
            "bk": np.ascontiguousarray(bk[cols]),
            "bv": np.ascontiguousarray(bv[cols]),
            "mask": np.ascontiguousarray(
                np.broadcast_to(attention_mask[b, 0, 0], (S,))),
        })
    res = run_bass_kernel_spmd(nc, in_maps, list(range(8)), **run_kw)
    out = np.empty((2, S, DM), dtype=np.float32)
    for c in range(8):
        b, g = c // 4, c % 4
        out[b, :, 192 * g:192 * (g + 1)] = res.results[c]["out"]
    if run_kw:
        return out, res
    return out
